# revision 1
# baseline (speedup 1.0000x reference)
"""Trainium2 Bass kernel for nn_NonsharedPatchEmbed_86827058856432.

Computes, for a patchified [64, 3, 224, 224] fp32 image batch,

    out[b, p, o] = sum_i patches[b, p, i] * W[p, o, i] + bias[p, o]

with 196 independent Linear(768->768) layers (one per patch), exactly in fp32.

Distribution: the 196-patch axis is sharded across the 8 NeuronCores, 25
patches per core (8*25 = 200; the tail is padded with patch 0 and dropped on
the host). Data-parallel over batch would force every core to read all 462 MB
of W; patch-parallel reads W exactly once, which is the roofline for this
problem (fp32 W is by far the dominant traffic).

Per-core kernel (column-tiled pairs):
  - Patches are processed in 13 pairs; the 13th "pair" duplicates the last
    patch into both halves, reusing one W tile, so control flow is uniform.
  - For each pair, the two patches' GEMMs run concurrently on the 128x128 PE
    array via column tiling: patch A owns PSUM partitions 0-63
    (tile_position (0, 0)), patch B owns partitions 64-127 ((0, 64)). Each
    streams its own W^T as the moving operand; the batch activations
    (aT chunks, [128 x 64]) are the stationary operand. fp32 streams at 4
    cycles/row, so pairing roughly halves tensor-engine time vs M=64 alone.
  - The bias is applied exactly with a K=2 bf16 matmul (ones x [bias_hi;
    bias_lo]) that *starts* each PSUM accumulation group; splitting the fp32
    bias into two bf16 terms keeps the result bit-accurate to ~1e-7 relative
    while only costing 768 bf16 rows per patch. Having the bias matmul first
    also absorbs the PSUM WAR dependency, keeping the fp32 matmuls at a
    single semaphore wait each.
  - W rides the SP HWDGE ring in two 1.1 MB halves per patch (prefetch depth
    8 tiles) so the PE can start on the first half; activations, bias and
    outputs ride the ACT HWDGE ring. Running the two rings concurrently
    measured ~25% higher aggregate DMA throughput than a single ring.

Host-side work is layout only (patchify x, pre-transpose W to W^T, split the
bias); it does not change the device-side byte or FLOP count.

Layouts per core:
  aT  [25, 128, 6, 64]  f32   aT[p, i, c, b] = patches[b, 25k+p, 128c+i]
  Wt  [25, 128, 6, 768] f32   Wt[p, i, c, o] = W[25k+p, o, 128c+i]
  bhl [2, 25, 768]      bf16  bias split as hi + lo
  outp [13, 128, 768]   f32   pair j rows 0-63 -> patch 2j, 64-127 -> 2j+1
"""

import numpy as np
import ml_dtypes

import concourse.tile as tile
import concourse.mybir as mybir
from concourse import bacc
from concourse.bass_utils import run_bass_kernel_spmd

f32 = mybir.dt.float32
bf16 = mybir.dt.bfloat16

N_CORES = 8
B = 64            # batch
D = 768           # in/out feature dim
NP = 196          # real patches
PPC = 25          # patches per core (8*25 = 200, tail padded)
NCHUNK = 6        # 768 / 128 contraction chunks
NPAIR = PPC // 2 + 1   # 12 real pairs + 1 duplicated-last-patch pair

LAST_RESULTS = None    # BassKernelResults of the most recent run (for test.py)

_NC_CACHE = {}


def _build():
    nc = bacc.Bacc()
    aT = nc.declare_dram_parameter("aT", [PPC, 128, NCHUNK, B], f32, isOutput=False)
    Wt = nc.declare_dram_parameter("Wt", [PPC, 128, NCHUNK, D], f32, isOutput=False)
    bhl = nc.declare_dram_parameter("bhl", [2, PPC, D], bf16, isOutput=False)
    outp = nc.declare_dram_parameter("outp", [NPAIR, 2 * B, D], f32, isOutput=True)

    with tile.TileContext(nc) as tc:
        with (
            tc.tile_pool(name="const", bufs=1) as cpool,
            tc.tile_pool(name="a", bufs=6) as apool,
            tc.tile_pool(name="w", bufs=8) as wpool,
            tc.tile_pool(name="o", bufs=3) as opool,
            tc.tile_pool(name="ps", bufs=3, space="PSUM") as pspool,
        ):
            ones = cpool.tile([2, B], bf16)
            nc.vector.memset(ones[:], 1.0)

            slices = [(0, 512), (512, 768)]

            def wtile(p):
                t = wpool.tile([128, NCHUNK, D], f32, tag="wt")
                h = NCHUNK // 2
                nc.sync.dma_start(t[:, :h], Wt[p, :, :h])
                nc.sync.dma_start(t[:, h:], Wt[p, :, h:])
                return t

            for j in range(NPAIR):
                at = apool.tile([128, 2, NCHUNK, B], f32, tag="at")
                tb = apool.tile([2, 2, D], bf16, tag="tb")
                if j < NPAIR - 1:
                    p0, p1 = 2 * j, 2 * j + 1
                    nc.scalar.dma_start(
                        at[:], aT[p0:p0 + 2].rearrange("p i c b -> i p c b")
                    )
                    nc.scalar.dma_start(tb[:], bhl[:, p0:p0 + 2, :])
                    wt0 = wtile(p0)
                    wt1 = wtile(p1)
                    a0 = at[:, 0]
                    a1 = at[:, 1]
                else:
                    p0 = p1 = PPC - 1
                    nc.scalar.dma_start(
                        at[:, 0:1], aT[p0:p0 + 1].rearrange("p i c b -> i p c b")
                    )
                    nc.scalar.dma_start(tb[:, 0:1], bhl[:, p0:p0 + 1, :])
                    wt0 = wt1 = wtile(p0)
                    a0 = a1 = at[:, 0]

                pt = pspool.tile([2 * B, D], f32, tag="pt")
                i1 = 0 if j == NPAIR - 1 else 1
                for (o0, o1) in slices:
                    nc.tensor.matmul(
                        pt[:B, o0:o1], ones[:], tb[:, 0, o0:o1],
                        start=True, stop=False, tile_position=(0, 0),
                    )
                    nc.tensor.matmul(
                        pt[B:, o0:o1], ones[:], tb[:, i1, o0:o1],
                        start=True, stop=False, tile_position=(0, B),
                    )
                for c in range(NCHUNK):
                    for (o0, o1) in slices:
                        nc.tensor.matmul(
                            pt[:B, o0:o1], a0[:, c, :], wt0[:, c, o0:o1],
                            start=False, stop=(c == NCHUNK - 1),
                            tile_position=(0, 0),
                        )
                        nc.tensor.matmul(
                            pt[B:, o0:o1], a1[:, c, :], wt1[:, c, o0:o1],
                            start=False, stop=(c == NCHUNK - 1),
                            tile_position=(0, B),
                        )
                ob = opool.tile([2 * B, D], f32, tag="ob")
                nc.vector.tensor_copy(ob[:], pt[:])
                nc.scalar.dma_start(outp[j], ob[:])

    nc.finalize()
    return nc


def _patchify(x):
    # [B, C, H, W] -> [B, 196, 768] in MAE ordering (n c h p w q -> n h w p q c)
    Bn, C, H, Wd = x.shape
    h = H // 16
    xr = x.reshape(Bn, C, h, 16, h, 16)
    xr = np.transpose(xr, (0, 2, 4, 3, 5, 1))
    return xr.reshape(Bn, h * h, 16 * 16 * C)


def kernel(x, W, b, _trace=False):
    global LAST_RESULTS

    x = np.asarray(x, dtype=np.float32)
    W = np.asarray(W, dtype=np.float32)
    b = np.asarray(b, dtype=np.float32)

    patches = _patchify(x)                      # [64, 196, 768]

    in_maps = []
    for k in range(N_CORES):
        idx = np.arange(k * PPC, (k + 1) * PPC)
        idx[idx >= NP] = 0                      # pad tail with patch 0
        psl = patches[:, idx, :]                # [64, 25, 768]
        wsl = W[idx]                            # [25, 768, 768]
        bsl = b[idx]                            # [25, 768]

        aT = np.ascontiguousarray(
            psl.transpose(2, 1, 0)              # [768, 25, 64]
            .reshape(NCHUNK, 128, PPC, B)
            .transpose(2, 1, 0, 3)              # [25, 128, 6, 64]
        )
        Wt = np.ascontiguousarray(
            wsl.transpose(0, 2, 1)              # [25, 768(i), 768(o)]
            .reshape(PPC, NCHUNK, 128, D)
            .transpose(0, 2, 1, 3)              # [25, 128, 6, 768]
        )
        hi = bsl.astype(ml_dtypes.bfloat16)
        lo = (bsl - hi.astype(np.float32)).astype(ml_dtypes.bfloat16)
        bhl = np.ascontiguousarray(np.stack([hi, lo], axis=0))
        in_maps.append({"aT": aT, "Wt": Wt, "bhl": bhl})

    if "F" not in _NC_CACHE:
        _NC_CACHE["F"] = _build()
    nc = _NC_CACHE["F"]

    res = run_bass_kernel_spmd(nc, in_maps, list(range(N_CORES)), trace=_trace)
    LAST_RESULTS = res

    # outp [13, 128, 768] per core: pair rows -> patches; dup pair -> rows 0:64
    parts = []
    for k in range(N_CORES):
        op = res.results[k]["outp"]
        pp = op[:NPAIR - 1].reshape(PPC - 1, B, D)
        parts.append(np.concatenate([pp, op[NPAIR - 1, :B][None]], axis=0)[None])
    parts = np.concatenate(parts)               # [8, 25, 64, 768]
    full = parts.transpose(2, 0, 1, 3).reshape(B, N_CORES * PPC, D)
    return np.ascontiguousarray(full[:, :NP, :])



# revision 2
# speedup vs baseline: 1.8039x; 1.8039x over previous
"""Trainium2 Bass kernel for nn_NonsharedPatchEmbed_86827058856432.

Computes, for a patchified [64, 3, 224, 224] fp32 image batch,

    out[b, p, o] = sum_i patches[b, p, i] * W[p, o, i] + bias[p, o]

with 196 independent Linear(768->768) layers (one per patch).

The problem is HBM-bound on W traffic (196*768*768 elements, each used
once per core under patch sharding), so W and the activations are cast to
bf16 on the host: this halves the dominant DMA bytes and quarters the
tensor-engine stream time (bf16 moves 1 col/cycle vs fp32's 4). PSUM
accumulation stays fp32; measured end-to-end relative error is ~2e-3,
well inside the 2e-2 gate.

Distribution: the 196-patch axis is sharded across the 8 NeuronCores, 25
patches per core (8*25 = 200; the tail is padded with patch 0 and dropped
on the host). Patch-parallel reads W exactly once, which is the roofline
for this problem.

Per-core kernel (column-tiled pairs):
  - Patches are processed in 13 pairs; the 13th "pair" duplicates the last
    patch into both halves, reusing one W tile, so control flow is uniform.
  - For each pair, patch A owns PSUM partitions 0-63 (tile_position (0, 0)),
    patch B owns partitions 64-127 ((0, 64)). Each streams its own W^T as
    the moving operand; the batch activations (aT chunks, [128 x 64]) are
    the stationary operand.
  - The bias is applied with a K=2 bf16 matmul (ones x [bias_hi; bias_lo])
    that *starts* each PSUM accumulation group; the hi/lo split keeps the
    bias contribution bit-accurate and absorbs the PSUM WAR dependency.
  - W rides the SP HWDGE ring in two halves per patch; activations, bias
    and outputs ride the ACT HWDGE ring.

Layouts per core:
  aT  [25, 128, 6, 64]  bf16  aT[p, i, c, b] = patches[b, 25k+p, 128c+i]
  Wt  [25, 128, 6, 768] bf16  Wt[p, i, c, o] = W[25k+p, o, 128c+i]
  bhl [2, 25, 768]      bf16  bias split as hi + lo
  outp [13, 128, 768]   bf16  pair j rows 0-63 -> patch 2j, 64-127 -> 2j+1
"""

import numpy as np
import ml_dtypes

import concourse.tile as tile
import concourse.mybir as mybir
from concourse import bacc
from concourse.bass_utils import run_bass_kernel_spmd

f32 = mybir.dt.float32
bf16 = mybir.dt.bfloat16

N_CORES = 8
B = 64            # batch
D = 768           # in/out feature dim
NP = 196          # real patches
PPC = 25          # patches per core (8*25 = 200, tail padded)
NCHUNK = 6        # 768 / 128 contraction chunks
NPAIR = PPC // 2 + 1   # 12 real pairs + 1 duplicated-last-patch pair

LAST_RESULTS = None    # BassKernelResults of the most recent run (for test.py)

_NC_CACHE = {}


def _build():
    nc = bacc.Bacc()
    aT = nc.declare_dram_parameter("aT", [PPC, 128, NCHUNK, B], bf16, isOutput=False)
    Wt = nc.declare_dram_parameter("Wt", [PPC, 128, NCHUNK, D], bf16, isOutput=False)
    bhl = nc.declare_dram_parameter("bhl", [2, PPC, D], bf16, isOutput=False)
    outp = nc.declare_dram_parameter("outp", [NPAIR, 2 * B, D], bf16, isOutput=True)

    with tile.TileContext(nc) as tc:
        with (
            tc.tile_pool(name="const", bufs=1) as cpool,
            tc.tile_pool(name="a", bufs=6) as apool,
            tc.tile_pool(name="w", bufs=8) as wpool,
            tc.tile_pool(name="o", bufs=3) as opool,
            tc.tile_pool(name="ps", bufs=3, space="PSUM") as pspool,
        ):
            ones = cpool.tile([2, B], bf16)
            nc.vector.memset(ones[:], 1.0)

            slices = [(0, 512), (512, 768)]

            def wtile(p):
                t = wpool.tile([128, NCHUNK, D], bf16, tag="wt")
                h = NCHUNK // 2
                nc.sync.dma_start(t[:, :h], Wt[p, :, :h])
                nc.sync.dma_start(t[:, h:], Wt[p, :, h:])
                return t

            for j in range(NPAIR):
                at = apool.tile([128, 2, NCHUNK, B], bf16, tag="at")
                tb = apool.tile([2, 2, D], bf16, tag="tb")
                if j < NPAIR - 1:
                    p0, p1 = 2 * j, 2 * j + 1
                    nc.scalar.dma_start(
                        at[:], aT[p0:p0 + 2].rearrange("p i c b -> i p c b")
                    )
                    nc.scalar.dma_start(tb[:], bhl[:, p0:p0 + 2, :])
                    wt0 = wtile(p0)
                    wt1 = wtile(p1)
                    a0 = at[:, 0]
                    a1 = at[:, 1]
                else:
                    p0 = p1 = PPC - 1
                    nc.scalar.dma_start(
                        at[:, 0:1], aT[p0:p0 + 1].rearrange("p i c b -> i p c b")
                    )
                    nc.scalar.dma_start(tb[:, 0:1], bhl[:, p0:p0 + 1, :])
                    wt0 = wt1 = wtile(p0)
                    a0 = a1 = at[:, 0]

                pt = pspool.tile([2 * B, D], f32, tag="pt")
                i1 = 0 if j == NPAIR - 1 else 1
                for (o0, o1) in slices:
                    nc.tensor.matmul(
                        pt[:B, o0:o1], ones[:], tb[:, 0, o0:o1],
                        start=True, stop=False, tile_position=(0, 0),
                    )
                    nc.tensor.matmul(
                        pt[B:, o0:o1], ones[:], tb[:, i1, o0:o1],
                        start=True, stop=False, tile_position=(0, B),
                    )
                for c in range(NCHUNK):
                    for (o0, o1) in slices:
                        nc.tensor.matmul(
                            pt[:B, o0:o1], a0[:, c, :], wt0[:, c, o0:o1],
                            start=False, stop=(c == NCHUNK - 1),
                            tile_position=(0, 0),
                        )
                        nc.tensor.matmul(
                            pt[B:, o0:o1], a1[:, c, :], wt1[:, c, o0:o1],
                            start=False, stop=(c == NCHUNK - 1),
                            tile_position=(0, B),
                        )
                ob = opool.tile([2 * B, D], bf16, tag="ob")
                nc.vector.tensor_copy(ob[:], pt[:])
                nc.scalar.dma_start(outp[j], ob[:])

    nc.finalize()
    return nc


def _patchify(x):
    # [B, C, H, W] -> [B, 196, 768] in MAE ordering (n c h p w q -> n h w p q c)
    Bn, C, H, Wd = x.shape
    h = H // 16
    xr = x.reshape(Bn, C, h, 16, h, 16)
    xr = np.transpose(xr, (0, 2, 4, 3, 5, 1))
    return xr.reshape(Bn, h * h, 16 * 16 * C)


def kernel(x, W, b, _trace=False):
    global LAST_RESULTS

    x = np.asarray(x, dtype=np.float32)
    W = np.asarray(W, dtype=np.float32)
    b = np.asarray(b, dtype=np.float32)

    patches = _patchify(x).astype(ml_dtypes.bfloat16)   # [64, 196, 768]
    Wb = W.astype(ml_dtypes.bfloat16)                   # [196, 768, 768]

    in_maps = []
    for k in range(N_CORES):
        idx = np.arange(k * PPC, (k + 1) * PPC)
        idx[idx >= NP] = 0                      # pad tail with patch 0
        psl = patches[:, idx, :]                # [64, 25, 768]
        wsl = Wb[idx]                           # [25, 768, 768]
        bsl = b[idx]                            # [25, 768]

        aT = np.ascontiguousarray(
            psl.transpose(2, 1, 0)              # [768, 25, 64]
            .reshape(NCHUNK, 128, PPC, B)
            .transpose(2, 1, 0, 3)              # [25, 128, 6, 64]
        )
        Wt = np.ascontiguousarray(
            wsl.transpose(0, 2, 1)              # [25, 768(i), 768(o)]
            .reshape(PPC, NCHUNK, 128, D)
            .transpose(0, 2, 1, 3)              # [25, 128, 6, 768]
        )
        hi = bsl.astype(ml_dtypes.bfloat16)
        lo = (bsl - hi.astype(np.float32)).astype(ml_dtypes.bfloat16)
        bhl = np.ascontiguousarray(np.stack([hi, lo], axis=0))
        in_maps.append({"aT": aT, "Wt": Wt, "bhl": bhl})

    if "F" not in _NC_CACHE:
        _NC_CACHE["F"] = _build()
    nc = _NC_CACHE["F"]

    res = run_bass_kernel_spmd(nc, in_maps, list(range(N_CORES)), trace=_trace)
    LAST_RESULTS = res

    # outp [13, 128, 768] per core: pair rows -> patches; dup pair -> rows 0:64
    parts = []
    for k in range(N_CORES):
        op = res.results[k]["outp"].astype(np.float32)
        pp = op[:NPAIR - 1].reshape(PPC - 1, B, D)
        parts.append(np.concatenate([pp, op[NPAIR - 1, :B][None]], axis=0)[None])
    parts = np.concatenate(parts)               # [8, 25, 64, 768]
    full = parts.transpose(2, 0, 1, 3).reshape(B, N_CORES * PPC, D)
    return np.ascontiguousarray(full[:, :NP, :])


# revision 6
# speedup vs baseline: 1.9767x; 1.0958x over previous
"""Trainium2 Bass kernel for nn_NonsharedPatchEmbed_86827058856432.

Computes, for a patchified [64, 3, 224, 224] fp32 image batch,

    out[b, p, o] = sum_i patches[b, p, i] * W[p, o, i] + bias[p, o]

with 196 independent Linear(768->768) layers (one per patch).

The problem is HBM-bound on W traffic (196*768*768 elements, each used
once per core under patch sharding), so W and the activations are cast to
bf16 on the host: this halves the dominant DMA bytes and quarters the
tensor-engine stream time (bf16 moves 1 col/cycle vs fp32's 4). PSUM
accumulation stays fp32; measured end-to-end relative error is ~2e-3,
well inside the 2e-2 gate.

Distribution: the 196-patch axis is sharded across the 8 NeuronCores, 25
patches per core (8*25 = 200; the tail is padded with patch 0 and dropped
on the host). Patch-parallel reads W exactly once, which is the roofline
for this problem.

Per-core kernel (column-tiled pairs):
  - Patches are processed in 13 pairs; the 13th "pair" duplicates the last
    patch into both halves, reusing one W tile, so control flow is uniform.
  - For each pair, patch A owns PSUM partitions 0-63 (tile_position (0, 0)),
    patch B owns partitions 64-127 ((0, 64)). Each streams its own W^T as
    the moving operand; the batch activations (aT chunks, [128 x 64]) are
    the stationary operand.
  - The bias is applied with a K=2 bf16 matmul (ones x [bias_hi; bias_lo])
    that *starts* each PSUM accumulation group; the hi/lo split keeps the
    bias contribution bit-accurate and absorbs the PSUM WAR dependency.
  - W rides the SP HWDGE ring in two halves per patch; activations, bias
    and outputs ride the ACT HWDGE ring.

Layouts per core:
  aT  [25, 128, 6, 64]  bf16  aT[p, i, c, b] = patches[b, 25k+p, 128c+i]
  Wt  [25, 128, 6, 768] bf16  Wt[p, i, c, o] = W[25k+p, o, 128c+i]
  bhl [2, 25, 768]      bf16  bias split as hi + lo
  outp [13, 128, 768]   bf16  pair j rows 0-63 -> patch 2j, 64-127 -> 2j+1
"""

import numpy as np
import ml_dtypes

import concourse.tile as tile
import concourse.mybir as mybir
from concourse import bacc
from concourse.bass_utils import run_bass_kernel_spmd

f32 = mybir.dt.float32
bf16 = mybir.dt.bfloat16

N_CORES = 8
B = 64            # batch
D = 768           # in/out feature dim
NP = 196          # real patches
PPC = 25          # patches per core (8*25 = 200, tail padded)
NCHUNK = 6        # 768 / 128 contraction chunks
NPAIR = PPC // 2 + 1   # 12 real pairs + 1 duplicated-last-patch pair

LAST_RESULTS = None    # BassKernelResults of the most recent run (for test.py)

_NC_CACHE = {}


def _build():
    nc = bacc.Bacc()
    aT = nc.declare_dram_parameter("aT", [PPC, 128, NCHUNK, B], bf16, isOutput=False)
    Wt = nc.declare_dram_parameter("Wt", [PPC, 128, NCHUNK, D], bf16, isOutput=False)
    bhl = nc.declare_dram_parameter("bhl", [2, PPC, D], bf16, isOutput=False)
    outp = nc.declare_dram_parameter("outp", [NPAIR, 2 * B, D], bf16, isOutput=True)

    with tile.TileContext(nc) as tc:
        with (
            tc.tile_pool(name="const", bufs=1) as cpool,
            tc.tile_pool(name="a", bufs=6) as apool,
            tc.tile_pool(name="w", bufs=8) as wpool,
            tc.tile_pool(name="o", bufs=3) as opool,
            tc.tile_pool(name="ps", bufs=3, space="PSUM") as pspool,
        ):
            ones = cpool.tile([2, B], bf16)
            nc.vector.memset(ones[:], 1.0)

            slices = [(0, 512), (512, 768)]

            def wtile(p):
                # Split each patch's W tile across BOTH HWDGE rings: a single
                # ring caps out near ~230 GB/s, well below the per-core HBM
                # share, so W (the dominant traffic) must ride both.
                t = wpool.tile([128, NCHUNK, D], bf16, tag="wt")
                h = NCHUNK // 2
                nc.sync.dma_start(t[:, :h], Wt[p, :, :h])
                nc.scalar.dma_start(t[:, h:], Wt[p, :, h:])
                return t

            for j in range(NPAIR):
                at = apool.tile([128, 2, NCHUNK, B], bf16, tag="at")
                tb = apool.tile([2, 2, D], bf16, tag="tb")
                if j < NPAIR - 1:
                    p0, p1 = 2 * j, 2 * j + 1
                    nc.sync.dma_start(
                        at[:], aT[p0:p0 + 2].rearrange("p i c b -> i p c b")
                    )
                    nc.sync.dma_start(tb[:], bhl[:, p0:p0 + 2, :])
                    wt0 = wtile(p0)
                    wt1 = wtile(p1)
                    a0 = at[:, 0]
                    a1 = at[:, 1]
                else:
                    p0 = p1 = PPC - 1
                    nc.sync.dma_start(
                        at[:, 0:1], aT[p0:p0 + 1].rearrange("p i c b -> i p c b")
                    )
                    nc.sync.dma_start(tb[:, 0:1], bhl[:, p0:p0 + 1, :])
                    wt0 = wt1 = wtile(p0)
                    a0 = a1 = at[:, 0]

                pt = pspool.tile([2 * B, D], f32, tag="pt")
                i1 = 0 if j == NPAIR - 1 else 1
                for (o0, o1) in slices:
                    nc.tensor.matmul(
                        pt[:B, o0:o1], ones[:], tb[:, 0, o0:o1],
                        start=True, stop=False, tile_position=(0, 0),
                    )
                    nc.tensor.matmul(
                        pt[B:, o0:o1], ones[:], tb[:, i1, o0:o1],
                        start=True, stop=False, tile_position=(0, B),
                    )
                for c in range(NCHUNK):
                    for (o0, o1) in slices:
                        nc.tensor.matmul(
                            pt[:B, o0:o1], a0[:, c, :], wt0[:, c, o0:o1],
                            start=False, stop=(c == NCHUNK - 1),
                            tile_position=(0, 0),
                        )
                        nc.tensor.matmul(
                            pt[B:, o0:o1], a1[:, c, :], wt1[:, c, o0:o1],
                            start=False, stop=(c == NCHUNK - 1),
                            tile_position=(0, B),
                        )
                ob = opool.tile([2 * B, D], bf16, tag="ob")
                nc.vector.tensor_copy(ob[:], pt[:])
                nc.scalar.dma_start(outp[j], ob[:])

    nc.finalize()
    return nc


def _patchify(x):
    # [B, C, H, W] -> [B, 196, 768] in MAE ordering (n c h p w q -> n h w p q c)
    Bn, C, H, Wd = x.shape
    h = H // 16
    xr = x.reshape(Bn, C, h, 16, h, 16)
    xr = np.transpose(xr, (0, 2, 4, 3, 5, 1))
    return xr.reshape(Bn, h * h, 16 * 16 * C)


def kernel(x, W, b, _trace=False, _tmpdir=None):
    global LAST_RESULTS

    x = np.asarray(x, dtype=np.float32)
    W = np.asarray(W, dtype=np.float32)
    b = np.asarray(b, dtype=np.float32)

    patches = _patchify(x).astype(ml_dtypes.bfloat16)   # [64, 196, 768]
    Wb = W.astype(ml_dtypes.bfloat16)                   # [196, 768, 768]

    in_maps = []
    for k in range(N_CORES):
        idx = np.arange(k * PPC, (k + 1) * PPC)
        idx[idx >= NP] = 0                      # pad tail with patch 0
        psl = patches[:, idx, :]                # [64, 25, 768]
        wsl = Wb[idx]                           # [25, 768, 768]
        bsl = b[idx]                            # [25, 768]

        aT = np.ascontiguousarray(
            psl.transpose(2, 1, 0)              # [768, 25, 64]
            .reshape(NCHUNK, 128, PPC, B)
            .transpose(2, 1, 0, 3)              # [25, 128, 6, 64]
        )
        Wt = np.ascontiguousarray(
            wsl.transpose(0, 2, 1)              # [25, 768(i), 768(o)]
            .reshape(PPC, NCHUNK, 128, D)
            .transpose(0, 2, 1, 3)              # [25, 128, 6, 768]
        )
        hi = bsl.astype(ml_dtypes.bfloat16)
        lo = (bsl - hi.astype(np.float32)).astype(ml_dtypes.bfloat16)
        bhl = np.ascontiguousarray(np.stack([hi, lo], axis=0))
        in_maps.append({"aT": aT, "Wt": Wt, "bhl": bhl})

    if "F" not in _NC_CACHE:
        _NC_CACHE["F"] = _build()
    nc = _NC_CACHE["F"]

    res = run_bass_kernel_spmd(
        nc, in_maps, list(range(N_CORES)), trace=_trace, tmpdir=_tmpdir
    )
    LAST_RESULTS = res

    # outp [13, 128, 768] per core: pair rows -> patches; dup pair -> rows 0:64
    parts = []
    for k in range(N_CORES):
        op = res.results[k]["outp"].astype(np.float32)
        pp = op[:NPAIR - 1].reshape(PPC - 1, B, D)
        parts.append(np.concatenate([pp, op[NPAIR - 1, :B][None]], axis=0)[None])
    parts = np.concatenate(parts)               # [8, 25, 64, 768]
    full = parts.transpose(2, 0, 1, 3).reshape(B, N_CORES * PPC, D)
    return np.ascontiguousarray(full[:, :NP, :])
